# revision 19
# baseline (speedup 1.0000x reference)
"""Multi-head attention (B=2, N=4096, D=768, H=12) on 8 Trainium2 NeuronCores.

Sharding: core c handles batch b = c//4 and heads [3g, 3g+1, 3g+2] with
g = c%4 (data parallel on B, head parallel on H). Each core computes its
heads' Q/K/V from x[b], runs softmax attention, and produces the partial
output projection for its head block; the host sums the 4 partials per
batch (row-parallel unshard) and adds b_proj.

Device kernel (v4 — fp8 probs, DoubleRow PV, dual-engine exp):
  - x is cast to bf16 and transposed via the DMA xbar into x^T chunks
    (x loads on the HWDGE scalar ring; transposes on the sync ring);
    QKV projection matmuls run in bf16 with the 6 output groups
    interleaved k-outer across 6 PSUM banks so drains overlap fills.
  - scores are built as S^T[k, q] blocks in bf16; the paired heads run
    their score matmuls concurrently in separate PE row groups; the
    third head is self-paired across even/odd key chunks.
  - exp is split across the scalar engine (ACT table exp -> fp8) and
    the vector engine (Schraudolph bit trick: e4m3_bits(2^y) ~= 8*y+56
    via one tensor_scalar with int8 output viewed as fp8e4). The bit
    trick's ±7% sawtooth is zero-mean and cancels between softmax
    numerator and denominator.
  - probabilities are fp8_e4m3; PV matmuls use DoubleRow perf mode
    contracting two 128-key chunks at once (2 fp8 MACs/cell/cycle),
    halving PE time. V (fp8, ones-augmented 65th column for the
    denominator) is layed out [128, pair, 2, 80] so the pair step is
    16B-aligned.
  - PV matmuls for query tile jq are emitted lagged ~2 chunk-pairs
    behind jq's score stream, so the PE always has ready work while
    the exp engines catch up (no PE idle -> HAM stays warm).
  - 1/denom: DVE reciprocal_approx_fast on the full [65,512] tile
    (base partition 0; custom DVE ops mis-execute at base!=0 on HW);
    the normalize multiplies run on GPSIMD and are deferred into the
    next query tile so the broadcast-DMA roundtrip stalls nothing.
"""

import numpy as np
import ml_dtypes
from contextlib import ExitStack

import concourse.bass as bass
from concourse import bacc
import concourse.tile as tile
import concourse.mybir as mybir
from concourse.bass_utils import run_bass_kernel_spmd

F32 = mybir.dt.float32
BF16 = mybir.dt.bfloat16
FP8 = mybir.dt.float8e4
I16 = mybir.dt.int16
I8 = mybir.dt.int8
AF = mybir.ActivationFunctionType
ALU = mybir.AluOpType
DR = mybir.MatmulPerfMode.DoubleRow

B, N, D, H, HD = 2, 4096, 768, 12, 64
SCALE = HD ** -0.5
NC = 8
NCHUNK = N // 128          # 32 key chunks of 128
NPAIR = NCHUNK // 2        # 16 key chunk-pairs (DoubleRow granularity)
NQT = N // 512             # 8 query tiles of 512
NSC = N // 512             # 8 seq chunks of 512 (QKV stage)
KCH = D // 128             # 6 contraction chunks

# Schraudolph fast-exp to fp8_e4m3: bits(exp(s*SCALE)) ~= s*C0 + C1.
LOG2E = 1.4426950408889634
EXP_C0 = 8.0 * SCALE * LOG2E
EXP_C1 = 56.0 - 0.33       # mean-centered sawtooth

# which exp tiles go to the vector engine (pattern repeats)
DVE_EXP_FRAC_PATTERN = (False, True, False, True, False, True, False)   # 3/7 on DVE


def build_program():
    nc = bacc.Bacc("TRN2", target_bir_lowering=False, debug=False)

    xb = nc.dram_tensor("xb", [N, D], BF16, kind="ExternalInput").ap()
    wg = nc.dram_tensor("wg", [128, 6, KCH, 128], BF16, kind="ExternalInput").ap()
    bias = nc.dram_tensor("bias", [128, 8], F32, kind="ExternalInput").ap()
    wpp = nc.dram_tensor("wpp", [128, D], BF16, kind="ExternalInput").ap()
    wp2 = nc.dram_tensor("wp2", [64, D], BF16, kind="ExternalInput").ap()
    y = nc.dram_tensor("y", [N, D], F32, kind="ExternalOutput").ap()

    with tile.TileContext(nc) as tc, ExitStack() as octx:
        const = octx.enter_context(tc.tile_pool(name="const", bufs=1))
        qkpool = octx.enter_context(tc.tile_pool(name="qk", bufs=1))
        vpool = octx.enter_context(tc.tile_pool(name="vaug", bufs=1))
        opool_sb = octx.enter_context(tc.tile_pool(name="onorm", bufs=1))

        bias_sb = const.tile([128, 8], F32)
        wpp_sb = const.tile([128, D], BF16)
        wp2_sb = const.tile([64, D], BF16)
        nc.scalar.dma_start(bias_sb[:], bias)
        nc.scalar.dma_start(wpp_sb[:], wpp)
        nc.scalar.dma_start(wp2_sb[:], wp2)

        # [hd, seq] layouts; pair heads stacked on partitions 0-63 / 64-127;
        # the h2 tensors hold the same head duplicated in both halves.
        QT_pair = qkpool.tile([128, N], BF16)
        KT_pair = qkpool.tile([128, N], BF16)
        QT_h2 = qkpool.tile([128, N], BF16)
        KT_h2 = qkpool.tile([128, N], BF16)

        # V natural [seq, hd] per head in fp8, DoubleRow pair layout
        # [128, pair, j, 80]: cols 0-63 = V, col 64 = ones (denominator),
        # cols 65-79 padding so the j step (80B) is 16B-aligned.
        V_aug = [
            vpool.tile([128, NPAIR, 2, 80], FP8, tag=f"vaug{h}", name=f"vaug{h}")
            for h in range(3)
        ]
        for h in range(3):
            nc.vector.memset(V_aug[h][:, :, :, 64], 1.0)

        # O^T (normalized) [feat, seq]: pair heads stacked; h2 separate.
        O_pair = opool_sb.tile([128, N], BF16)
        O_h2 = opool_sb.tile([64, N], BF16)

        # ------------- stage A+B: x^T (bf16) and QKV projection -------------
        with ExitStack() as bctx:
            wpool = bctx.enter_context(tc.tile_pool(name="wqkv", bufs=1))
            xpool = bctx.enter_context(tc.tile_pool(name="xin", bufs=4))
            xtpool = bctx.enter_context(tc.tile_pool(name="xT", bufs=4))
            vtpool = bctx.enter_context(tc.tile_pool(name="vt", bufs=4))
            qkvps = bctx.enter_context(tc.tile_pool(name="qkv", bufs=6, space="PSUM"))

            wsb = wpool.tile([128, 6, KCH, 128], BF16)
            nc.scalar.dma_start(wsb[:], wg)

            def make_xT(j):
                # xT_j[p, t, k, m] = x4[m, t, 128k+p]: one contiguous xbar
                # transpose per chunk (strided xbar outputs mis-write on HW);
                # x arrives bf16 from the host so no cast pass is needed
                xT_j = xtpool.tile([128, 4, KCH, 128], BF16, tag="xT", name=f"xT_{j}")
                x4 = xpool.tile([128, 4, D], BF16, tag="x_t", name=f"x_{j}")
                nc.gpsimd.dma_start(
                    x4[:],
                    xb[512 * j: 512 * (j + 1), :].rearrange("(t p) d -> p t d", p=128),
                )
                nc.sync.dma_start_transpose(
                    xT_j[:].rearrange("p t k m -> p (t k) m"),
                    x4[:].rearrange("p t d -> p (t d)"),
                )
                return xT_j

            def qkv_chunk(j, xT_j):
                jsl = bass.ts(j, 512)
                vt_p = vtpool.tile([128, 512], BF16, tag="vtp", name=f"vtp_{j}")
                vt_2 = vtpool.tile([64, 512], BF16, tag="vt2", name=f"vt2_{j}")
                # k-outer / g-inner: consecutive matmuls hit different PSUM
                # banks so each drain hides under the next matmul's fill
                pss = [
                    qkvps.tile([128, 512], F32, tag="ps", name=f"ps_{j}_{g}")
                    for g in range(6)
                ]
                for k in range(KCH):
                    for g in range(6):
                        nc.tensor.matmul(
                            pss[g][:], wsb[:, g, k, :], xT_j[:, :, k, :],
                            start=(k == 0), stop=(k == KCH - 1),
                        )
                dests = {0: QT_pair[:, jsl], 1: KT_pair[:, jsl], 3: QT_h2[:, jsl],
                         4: KT_h2[:, jsl], 2: vt_p[:]}
                for g in (2, 5, 0, 1, 3, 4):
                    ps = pss[g]
                    # bias-add + bf16 cast on DVE (per-partition bias scalar);
                    # keeps the scalar engine free for its DMA ring in A+B
                    if g == 5:  # single head, rows 0-63 only
                        nc.vector.tensor_scalar_add(vt_2[:], ps[0:64, :],
                                                    bias_sb[0:64, g: g + 1])
                    else:
                        nc.vector.tensor_scalar_add(dests[g], ps[:],
                                                    bias_sb[:, g: g + 1])

                # V^T -> V natural via xbar transpose (contiguous staging) then
                # strided DVE copy into the fp8 DoubleRow layout; chunks
                # c = 4j+t map to (pair, j2) = (2j + t//2, t%2) in order.
                # The pair heads transpose together in one [128,512] xbar pass.
                vstp = vtpool.tile([128, 4, 128], BF16, tag="vstp", name=f"vstp_{j}")
                nc.sync.dma_start_transpose(vstp[:], vt_p[:])
                vst2 = vtpool.tile([128, 4, 64], BF16, tag="vst2", name=f"vst2_{j}")
                nc.sync.dma_start_transpose(vst2[:], vt_2[:])
                for h, src in ((0, vstp[:, :, 0:64]), (1, vstp[:, :, 64:128]), (2, vst2[:])):
                    dst = V_aug[h][:, 2 * j: 2 * j + 2, :, 0:64]
                    nc.vector.tensor_copy(
                        dst.rearrange("p a b m -> p (a b) m"), src
                    )

            xts = {}
            for j in range(NSC):
                xts[j] = make_xT(j)
                if j >= 2:
                    qkv_chunk(j - 2, xts.pop(j - 2))
            qkv_chunk(NSC - 2, xts.pop(NSC - 2))
            qkv_chunk(NSC - 1, xts.pop(NSC - 1))

        # ---------------- stage C: attention ----------------
        with ExitStack() as cctx:
            spool = cctx.enter_context(tc.tile_pool(name="s", bufs=3, space="PSUM"))
            opool = cctx.enter_context(tc.tile_pool(name="o", bufs=2, space="PSUM"))
            ppool = cctx.enter_context(tc.tile_pool(name="p", bufs=8))
            osb_pool = cctx.enter_context(tc.tile_pool(name="osb", bufs=6))
            bcsb = cctx.enter_context(tc.tile_pool(name="bcs", bufs=4))
            rpool = cctx.enter_context(tc.tile_pool(name="r", bufs=4))
            rdpool = cctx.enter_context(tc.tile_pool(name="rd", bufs=4, space="DRAM"))
            ysb_pool = cctx.enter_context(tc.tile_pool(name="ysb", bufs=3))

            exp_idx = [0]

            def exp_tile(dst_fp8, src_ps):
                # dst_fp8: [128, 1024] fp8 view; src_ps: [128, 1024] PSUM f32
                i = exp_idx[0]
                exp_idx[0] += 1
                with nc.allow_low_precision(reason="fp8 softmax probs"):
                    if DVE_EXP_FRAC_PATTERN[i % len(DVE_EXP_FRAC_PATTERN)]:
                        nc.vector.tensor_scalar(
                            dst_fp8.bitcast(I8), src_ps, EXP_C0, EXP_C1,
                            ALU.mult, ALU.add,
                        )
                    else:
                        nc.scalar.activation(dst_fp8, src_ps, AF.Exp, scale=SCALE)

            # normalize phase 1: drain PSUM to SBUF, start 1/denom broadcast
            def normalize_start(o_ps, h, qsl):
                o_sb = osb_pool.tile([65, 512], F32)
                nc.vector.tensor_copy(o_sb[:], o_ps[:])
                # custom DVE ops mis-execute at base_partition != 0 on HW:
                # approx-reciprocal the whole [65,512] tile, use only row 64.
                r = rpool.tile([65, 512], F32)
                nc.vector.reciprocal_approx_fast(r[:], o_sb[:])
                rd = rdpool.tile([1, 512], F32)
                nc.gpsimd.dma_start(rd[:], r[64:65, :])
                bcs = bcsb.tile([64, 512], F32)
                nc.gpsimd.dma_start(bcs[:], rd[:].to_broadcast([64, 512]))
                return (o_sb, bcs, h, qsl)

            # normalize phase 2 (deferred; on the otherwise-idle GPSIMD)
            def normalize_finish(st):
                o_sb, bcs, h, qsl = st
                dest = O_pair[64 * h: 64 * (h + 1), qsl] if h < 2 else O_h2[:, qsl]
                nc.gpsimd.tensor_tensor(dest, o_sb[0:64, :], bcs[:], ALU.mult)

            def proj_subtile(pj, t4):
                # output projection of one 128-row q-subtile; borrows an s slot
                t = 4 * pj + t4
                tsl = bass.ts(t, 128)
                ysb = ysb_pool.tile([128, D], F32, tag="ysb", name=f"ysb_{t}")
                for half in range(2):
                    hsl = bass.ts(half, 384)
                    yp = spool.tile([128, 384], F32, tag="s2", name=f"yp_{t}_{half}")
                    nc.tensor.matmul(yp[:], O_pair[:, tsl], wpp_sb[:, hsl],
                                     start=True, stop=False)
                    nc.tensor.matmul(yp[:], O_h2[:, tsl], wp2_sb[:, hsl],
                                     start=False, stop=True)
                    nc.vector.tensor_copy(ysb[:, hsl], yp[:])
                nc.sync.dma_start(y[128 * t: 128 * (t + 1), :], ysb[:])

            pending = []          # query tiles awaiting projection
            pending_norm = []     # normalize finishes awaiting bcs DMA
            pv_q = []             # lagged PV matmul thunks

            def pump_pv(n=1):
                for _ in range(min(n, len(pv_q))):
                    pv_q.pop(0)()

            for jq in range(NQT):
                # drain the previous tile's lagged PV work first so its
                # normalizes are queued before this tile's pop points
                pump_pv(len(pv_q))
                qsl = bass.ts(jq, 512)
                o0 = opool.tile([65, 512], F32, tag="o", name=f"o0_{jq}")
                o1 = opool.tile([65, 512], F32, tag="o", name=f"o1_{jq}")

                # ---- heads h0/h1: 16 chunk-pairs ----
                for cc in range(NPAIR):
                    p4 = ppool.tile([128, 2, 2, 512], FP8, tag="p4",
                                    name=f"p4_{jq}_{cc}")
                    for j2 in (0, 1):
                        c = 2 * cc + j2
                        if pending_norm and c in (3, 5, 7, 9):
                            normalize_finish(pending_norm.pop(0))
                        if pending and pending[0] <= jq - 2:
                            if c in (10, 15, 20, 25):
                                # all of pj's normalizes must be emitted before
                                # its projection reads O_pair/O_h2
                                while pending_norm:
                                    normalize_finish(pending_norm.pop(0))
                                pj = pending[0]
                                proj_subtile(pj, (c - 10) // 5)
                                if c == 25:
                                    pending.pop(0)
                        ksl = bass.ts(c, 128)
                        s2 = spool.tile([128, 1024], F32)
                        nc.tensor.matmul(s2[:, 0:512], KT_pair[0:64, ksl],
                                         QT_pair[0:64, qsl], start=True, stop=True)
                        nc.tensor.matmul(s2[:, 512:1024], KT_pair[64:128, ksl],
                                         QT_pair[64:128, qsl], start=True, stop=True)
                        exp_tile(p4[:, j2].rearrange("p a b -> p (a b)"), s2[:])

                    def pv_pair(cc=cc, p4=p4, o0=o0, o1=o1, qsl=qsl):
                        st = (cc == 0)
                        sp = (cc == NPAIR - 1)
                        nc.tensor.matmul(o0[:], V_aug[0][:, cc, :, 0:65],
                                         p4[:, :, 0, :], start=st, stop=sp,
                                         perf_mode=DR)
                        nc.tensor.matmul(o1[:], V_aug[1][:, cc, :, 0:65],
                                         p4[:, :, 1, :], start=st, stop=sp,
                                         perf_mode=DR)
                        if sp:
                            pending_norm.append(normalize_start(o0, 0, qsl))
                            pending_norm.append(normalize_start(o1, 1, qsl))
                    pv_q.append(pv_pair)
                    if cc >= 2:
                        pump_pv()

                # ---- head h2: 16 chunk-pairs (even/odd in the row groups) ----
                o2 = opool.tile([65, 512], F32, tag="o", name=f"o2_{jq}")
                for cc in range(NPAIR):
                    ce, co = 2 * cc, 2 * cc + 1
                    s2 = spool.tile([128, 1024], F32)
                    nc.tensor.matmul(s2[:, 0:512], KT_h2[0:64, bass.ts(ce, 128)],
                                     QT_h2[0:64, qsl], start=True, stop=True)
                    nc.tensor.matmul(s2[:, 512:1024], KT_h2[64:128, bass.ts(co, 128)],
                                     QT_h2[64:128, qsl], start=True, stop=True)
                    p2h = ppool.tile([128, 2, 512], FP8, tag="p2h",
                                     name=f"p2h_{jq}_{cc}")
                    exp_tile(p2h[:].rearrange("p a b -> p (a b)"), s2[:])

                    def pv_h2(cc=cc, p2h=p2h, o2=o2, qsl=qsl):
                        st = (cc == 0)
                        sp = (cc == NPAIR - 1)
                        nc.tensor.matmul(o2[:], V_aug[2][:, cc, :, 0:65],
                                         p2h[:], start=st, stop=sp, perf_mode=DR)
                        if sp:
                            pending_norm.append(normalize_start(o2, 2, qsl))
                    pv_q.append(pv_h2)
                    pump_pv()

                pending.append(jq)

            pump_pv(len(pv_q))
            while pending_norm:
                normalize_finish(pending_norm.pop(0))
            for pj in pending:
                for t4 in range(4):
                    proj_subtile(pj, t4)

    nc.compile()
    return nc


_PROGRAM = None


def _get_program():
    global _PROGRAM
    if _PROGRAM is None:
        _PROGRAM = build_program()
    return _PROGRAM


def make_core_inputs(x, W_qkv, b_qkv, W_proj):
    """Per-core input dicts implementing the (batch, head-group) sharding."""
    x = np.ascontiguousarray(np.asarray(x, np.float32))
    W_qkv = np.asarray(W_qkv, np.float32)
    b_qkv = np.asarray(b_qkv, np.float32)
    W_proj = np.asarray(W_proj, np.float32)
    bf = ml_dtypes.bfloat16

    ins = []
    for c in range(NC):
        b, g = divmod(c, 4)
        col = 192 * g
        wgq = np.zeros((6, D, 128), np.float32)
        bias = np.zeros((128, 8), np.float32)
        for i, off in enumerate((0, D, 2 * D)):  # q, k, v blocks of W_qkv
            wgq[i] = W_qkv[:, off + col: off + col + 128]
            bias[:, i] = b_qkv[off + col: off + col + 128]
        q2 = W_qkv[:, col + 128: col + 192]
        k2 = W_qkv[:, D + col + 128: D + col + 192]
        v2 = W_qkv[:, 2 * D + col + 128: 2 * D + col + 192]
        wgq[3] = np.concatenate([q2, q2], axis=1)
        wgq[4] = np.concatenate([k2, k2], axis=1)
        wgq[5, :, 0:64] = v2
        bias[0:64, 3] = b_qkv[col + 128: col + 192]
        bias[64:128, 3] = bias[0:64, 3]
        bias[0:64, 4] = b_qkv[D + col + 128: D + col + 192]
        bias[64:128, 4] = bias[0:64, 4]
        bias[0:64, 5] = b_qkv[2 * D + col + 128: 2 * D + col + 192]

        wgt = np.ascontiguousarray(
            wgq.reshape(6, KCH, 128, 128).transpose(2, 0, 1, 3)
        )  # [g, (c p), m] -> [p, g, c, m]
        ins.append({
            "xb": np.ascontiguousarray(x[b]).astype(bf),
            "wg": wgt.astype(bf),
            "bias": bias,
            "wpp": W_proj[col: col + 128, :].astype(bf),
            "wp2": W_proj[col + 128: col + 192, :].astype(bf),
        })
    return ins


def gather_output(results, b_proj):
    b_proj = np.asarray(b_proj, np.float32)
    outs = []
    for b in range(B):
        acc = results[4 * b]["y"].astype(np.float32).copy()
        for c in range(4 * b + 1, 4 * b + 4):
            acc += results[c]["y"]
        outs.append(acc + b_proj)
    return np.stack(outs).astype(np.float32)


def kernel(x, W_qkv, b_qkv, W_proj, b_proj):
    ins = make_core_inputs(x, W_qkv, b_qkv, W_proj)
    prog = _get_program()
    res = run_bass_kernel_spmd(prog, ins, core_ids=list(range(NC)))
    return gather_output(res.results, b_proj)


# revision 20
# speedup vs baseline: 1.0053x; 1.0053x over previous
"""Multi-head attention (B=2, N=4096, D=768, H=12) on 8 Trainium2 NeuronCores.

Sharding: core c handles batch b = c//4 and heads [3g, 3g+1, 3g+2] with
g = c%4 (data parallel on B, head parallel on H). Each core computes its
heads' Q/K/V from x[b], runs softmax attention, and produces the partial
output projection for its head block; the host sums the 4 partials per
batch (row-parallel unshard) and adds b_proj.

Device kernel (v4 — fp8 probs, DoubleRow PV, dual-engine exp):
  - x is cast to bf16 and transposed via the DMA xbar into x^T chunks
    (x loads on the HWDGE scalar ring; transposes on the sync ring);
    QKV projection matmuls run in bf16 with the 6 output groups
    interleaved k-outer across 6 PSUM banks so drains overlap fills.
  - scores are built as S^T[k, q] blocks in bf16; the paired heads run
    their score matmuls concurrently in separate PE row groups; the
    third head is self-paired across even/odd key chunks.
  - exp is split across the scalar engine (ACT table exp -> fp8) and
    the vector engine (Schraudolph bit trick: e4m3_bits(2^y) ~= 8*y+56
    via one tensor_scalar with int8 output viewed as fp8e4). The bit
    trick's ±7% sawtooth is zero-mean and cancels between softmax
    numerator and denominator.
  - probabilities are fp8_e4m3; PV matmuls use DoubleRow perf mode
    contracting two 128-key chunks at once (2 fp8 MACs/cell/cycle),
    halving PE time. V (fp8, ones-augmented 65th column for the
    denominator) is layed out [128, pair, 2, 80] so the pair step is
    16B-aligned.
  - PV matmuls for query tile jq are emitted lagged ~2 chunk-pairs
    behind jq's score stream, so the PE always has ready work while
    the exp engines catch up (no PE idle -> HAM stays warm).
  - 1/denom: DVE reciprocal_approx_fast on the full [65,512] tile
    (base partition 0; custom DVE ops mis-execute at base!=0 on HW);
    the normalize multiplies run on GPSIMD and are deferred into the
    next query tile so the broadcast-DMA roundtrip stalls nothing.
"""

import numpy as np
import ml_dtypes
from contextlib import ExitStack

import concourse.bass as bass
from concourse import bacc
import concourse.tile as tile
import concourse.mybir as mybir
from concourse.bass_utils import run_bass_kernel_spmd

F32 = mybir.dt.float32
BF16 = mybir.dt.bfloat16
FP8 = mybir.dt.float8e4
I16 = mybir.dt.int16
I8 = mybir.dt.int8
AF = mybir.ActivationFunctionType
ALU = mybir.AluOpType
DR = mybir.MatmulPerfMode.DoubleRow

B, N, D, H, HD = 2, 4096, 768, 12, 64
SCALE = HD ** -0.5
NC = 8
NCHUNK = N // 128          # 32 key chunks of 128
NPAIR = NCHUNK // 2        # 16 key chunk-pairs (DoubleRow granularity)
NQT = N // 512             # 8 query tiles of 512
NSC = N // 512             # 8 seq chunks of 512 (QKV stage)
KCH = D // 128             # 6 contraction chunks

# Schraudolph fast-exp to fp8_e4m3: bits(exp(s*SCALE)) ~= s*C0 + C1.
LOG2E = 1.4426950408889634
EXP_C0 = 8.0 * SCALE * LOG2E
EXP_C1 = 56.0 - 0.33       # mean-centered sawtooth

# which exp tiles go to the vector engine (pattern repeats)
DVE_EXP_FRAC_PATTERN = (False, True, False, True, False)   # 2/5 on DVE


def build_program():
    nc = bacc.Bacc("TRN2", target_bir_lowering=False, debug=False)

    xb = nc.dram_tensor("xb", [N, D], BF16, kind="ExternalInput").ap()
    wg = nc.dram_tensor("wg", [128, 6, KCH, 128], BF16, kind="ExternalInput").ap()
    bias = nc.dram_tensor("bias", [128, 8], F32, kind="ExternalInput").ap()
    wpp = nc.dram_tensor("wpp", [128, D], BF16, kind="ExternalInput").ap()
    wp2 = nc.dram_tensor("wp2", [64, D], BF16, kind="ExternalInput").ap()
    y = nc.dram_tensor("y", [N, D], F32, kind="ExternalOutput").ap()

    with tile.TileContext(nc) as tc, ExitStack() as octx:
        const = octx.enter_context(tc.tile_pool(name="const", bufs=1))
        qkpool = octx.enter_context(tc.tile_pool(name="qk", bufs=1))
        vpool = octx.enter_context(tc.tile_pool(name="vaug", bufs=1))
        opool_sb = octx.enter_context(tc.tile_pool(name="onorm", bufs=1))

        bias_sb = const.tile([128, 8], F32)
        wpp_sb = const.tile([128, D], BF16)
        wp2_sb = const.tile([64, D], BF16)
        nc.scalar.dma_start(bias_sb[:], bias)
        nc.scalar.dma_start(wpp_sb[:], wpp)
        nc.scalar.dma_start(wp2_sb[:], wp2)

        # [hd, seq] layouts; pair heads stacked on partitions 0-63 / 64-127;
        # the h2 tensors hold the same head duplicated in both halves.
        QT_pair = qkpool.tile([128, N], BF16)
        KT_pair = qkpool.tile([128, N], BF16)
        QT_h2 = qkpool.tile([128, N], BF16)
        KT_h2 = qkpool.tile([128, N], BF16)

        # V natural [seq, hd] per head in fp8, DoubleRow pair layout
        # [128, pair, j, 80]: cols 0-63 = V, col 64 = ones (denominator),
        # cols 65-79 padding so the j step (80B) is 16B-aligned.
        V_aug = [
            vpool.tile([128, NPAIR, 2, 80], FP8, tag=f"vaug{h}", name=f"vaug{h}")
            for h in range(3)
        ]
        for h in range(3):
            nc.vector.memset(V_aug[h][:, :, :, 64], 1.0)

        # O^T (normalized) [feat, seq]: pair heads stacked; h2 separate.
        O_pair = opool_sb.tile([128, N], BF16)
        O_h2 = opool_sb.tile([64, N], BF16)

        # ------------- stage A+B: x^T (bf16) and QKV projection -------------
        with ExitStack() as bctx:
            wpool = bctx.enter_context(tc.tile_pool(name="wqkv", bufs=1))
            xpool = bctx.enter_context(tc.tile_pool(name="xin", bufs=4))
            xtpool = bctx.enter_context(tc.tile_pool(name="xT", bufs=4))
            vtpool = bctx.enter_context(tc.tile_pool(name="vt", bufs=4))
            qkvps = bctx.enter_context(tc.tile_pool(name="qkv", bufs=6, space="PSUM"))

            wsb = wpool.tile([128, 6, KCH, 128], BF16)
            nc.scalar.dma_start(wsb[:], wg)

            def make_xT(j):
                # xT_j[p, t, k, m] = x4[m, t, 128k+p]: one contiguous xbar
                # transpose per chunk (strided xbar outputs mis-write on HW);
                # x arrives bf16 from the host so no cast pass is needed
                xT_j = xtpool.tile([128, 4, KCH, 128], BF16, tag="xT", name=f"xT_{j}")
                x4 = xpool.tile([128, 4, D], BF16, tag="x_t", name=f"x_{j}")
                nc.gpsimd.dma_start(
                    x4[:],
                    xb[512 * j: 512 * (j + 1), :].rearrange("(t p) d -> p t d", p=128),
                )
                nc.sync.dma_start_transpose(
                    xT_j[:].rearrange("p t k m -> p (t k) m"),
                    x4[:].rearrange("p t d -> p (t d)"),
                )
                return xT_j

            def qkv_chunk(j, xT_j):
                jsl = bass.ts(j, 512)
                vt_p = vtpool.tile([128, 512], BF16, tag="vtp", name=f"vtp_{j}")
                vt_2 = vtpool.tile([64, 512], BF16, tag="vt2", name=f"vt2_{j}")
                # k-outer / g-inner: consecutive matmuls hit different PSUM
                # banks so each drain hides under the next matmul's fill
                pss = [
                    qkvps.tile([128, 512], F32, tag="ps", name=f"ps_{j}_{g}")
                    for g in range(6)
                ]
                for k in range(KCH):
                    for g in range(6):
                        nc.tensor.matmul(
                            pss[g][:], wsb[:, g, k, :], xT_j[:, :, k, :],
                            start=(k == 0), stop=(k == KCH - 1),
                        )
                dests = {0: QT_pair[:, jsl], 1: KT_pair[:, jsl], 3: QT_h2[:, jsl],
                         4: KT_h2[:, jsl], 2: vt_p[:]}
                for g in (2, 5, 0, 1, 3, 4):
                    ps = pss[g]
                    # bias-add + bf16 cast on DVE (per-partition bias scalar);
                    # keeps the scalar engine free for its DMA ring in A+B
                    if g == 5:  # single head, rows 0-63 only
                        nc.vector.tensor_scalar_add(vt_2[:], ps[0:64, :],
                                                    bias_sb[0:64, g: g + 1])
                    else:
                        nc.vector.tensor_scalar_add(dests[g], ps[:],
                                                    bias_sb[:, g: g + 1])

                # V^T -> V natural via xbar transpose (contiguous staging) then
                # strided DVE copy into the fp8 DoubleRow layout; chunks
                # c = 4j+t map to (pair, j2) = (2j + t//2, t%2) in order.
                # The pair heads transpose together in one [128,512] xbar pass.
                vstp = vtpool.tile([128, 4, 128], BF16, tag="vstp", name=f"vstp_{j}")
                nc.sync.dma_start_transpose(vstp[:], vt_p[:])
                vst2 = vtpool.tile([128, 4, 64], BF16, tag="vst2", name=f"vst2_{j}")
                nc.sync.dma_start_transpose(vst2[:], vt_2[:])
                for h, src in ((0, vstp[:, :, 0:64]), (1, vstp[:, :, 64:128]), (2, vst2[:])):
                    dst = V_aug[h][:, 2 * j: 2 * j + 2, :, 0:64]
                    nc.vector.tensor_copy(
                        dst.rearrange("p a b m -> p (a b) m"), src
                    )

            xts = {}
            for j in range(NSC):
                xts[j] = make_xT(j)
                if j >= 2:
                    qkv_chunk(j - 2, xts.pop(j - 2))
            qkv_chunk(NSC - 2, xts.pop(NSC - 2))
            qkv_chunk(NSC - 1, xts.pop(NSC - 1))

        # ---------------- stage C: attention ----------------
        with ExitStack() as cctx:
            spool = cctx.enter_context(tc.tile_pool(name="s", bufs=3, space="PSUM"))
            opool = cctx.enter_context(tc.tile_pool(name="o", bufs=2, space="PSUM"))
            ppool = cctx.enter_context(tc.tile_pool(name="p", bufs=8))
            osb_pool = cctx.enter_context(tc.tile_pool(name="osb", bufs=6))
            bcsb = cctx.enter_context(tc.tile_pool(name="bcs", bufs=4))
            rpool = cctx.enter_context(tc.tile_pool(name="r", bufs=4))
            rdpool = cctx.enter_context(tc.tile_pool(name="rd", bufs=4, space="DRAM"))
            ysb_pool = cctx.enter_context(tc.tile_pool(name="ysb", bufs=3))

            exp_idx = [0]

            def exp_tile(dst_fp8, src_ps):
                # dst_fp8: [128, 1024] fp8 view; src_ps: [128, 1024] PSUM f32
                i = exp_idx[0]
                exp_idx[0] += 1
                with nc.allow_low_precision(reason="fp8 softmax probs"):
                    if DVE_EXP_FRAC_PATTERN[i % len(DVE_EXP_FRAC_PATTERN)]:
                        nc.vector.tensor_scalar(
                            dst_fp8.bitcast(I8), src_ps, EXP_C0, EXP_C1,
                            ALU.mult, ALU.add,
                        )
                    else:
                        nc.scalar.activation(dst_fp8, src_ps, AF.Exp, scale=SCALE)

            # normalize phase 1: drain PSUM to SBUF, start 1/denom broadcast
            def normalize_start(o_ps, h, qsl):
                o_sb = osb_pool.tile([65, 512], F32)
                nc.vector.tensor_copy(o_sb[:], o_ps[:])
                # custom DVE ops mis-execute at base_partition != 0 on HW:
                # approx-reciprocal the whole [65,512] tile, use only row 64.
                r = rpool.tile([65, 512], F32)
                nc.vector.reciprocal_approx_fast(r[:], o_sb[:])
                rd = rdpool.tile([1, 512], F32)
                nc.gpsimd.dma_start(rd[:], r[64:65, :])
                bcs = bcsb.tile([64, 512], F32)
                nc.gpsimd.dma_start(bcs[:], rd[:].to_broadcast([64, 512]))
                return (o_sb, bcs, h, qsl)

            # normalize phase 2 (deferred; on the otherwise-idle GPSIMD)
            def normalize_finish(st):
                o_sb, bcs, h, qsl = st
                dest = O_pair[64 * h: 64 * (h + 1), qsl] if h < 2 else O_h2[:, qsl]
                nc.gpsimd.tensor_tensor(dest, o_sb[0:64, :], bcs[:], ALU.mult)

            def proj_subtile(pj, t4):
                # output projection of one 128-row q-subtile; borrows an s slot
                t = 4 * pj + t4
                tsl = bass.ts(t, 128)
                ysb = ysb_pool.tile([128, D], F32, tag="ysb", name=f"ysb_{t}")
                for half in range(2):
                    hsl = bass.ts(half, 384)
                    yp = spool.tile([128, 384], F32, tag="s2", name=f"yp_{t}_{half}")
                    nc.tensor.matmul(yp[:], O_pair[:, tsl], wpp_sb[:, hsl],
                                     start=True, stop=False)
                    nc.tensor.matmul(yp[:], O_h2[:, tsl], wp2_sb[:, hsl],
                                     start=False, stop=True)
                    nc.vector.tensor_copy(ysb[:, hsl], yp[:])
                nc.sync.dma_start(y[128 * t: 128 * (t + 1), :], ysb[:])

            pending = []          # query tiles awaiting projection
            pending_norm = []     # normalize finishes awaiting bcs DMA
            pv_q = []             # lagged PV matmul thunks

            def pump_pv(n=1):
                for _ in range(min(n, len(pv_q))):
                    pv_q.pop(0)()

            for jq in range(NQT):
                # drain the previous tile's lagged PV work first so its
                # normalizes are queued before this tile's pop points
                pump_pv(len(pv_q))
                qsl = bass.ts(jq, 512)
                o0 = opool.tile([65, 512], F32, tag="o", name=f"o0_{jq}")
                o1 = opool.tile([65, 512], F32, tag="o", name=f"o1_{jq}")

                # ---- heads h0/h1: 16 chunk-pairs ----
                for cc in range(NPAIR):
                    p4 = ppool.tile([128, 2, 2, 512], FP8, tag="p4",
                                    name=f"p4_{jq}_{cc}")
                    for j2 in (0, 1):
                        c = 2 * cc + j2
                        if pending_norm and c in (3, 5, 7, 9):
                            normalize_finish(pending_norm.pop(0))
                        if pending and pending[0] <= jq - 2:
                            if c in (10, 15, 20, 25):
                                # all of pj's normalizes must be emitted before
                                # its projection reads O_pair/O_h2
                                while pending_norm:
                                    normalize_finish(pending_norm.pop(0))
                                pj = pending[0]
                                proj_subtile(pj, (c - 10) // 5)
                                if c == 25:
                                    pending.pop(0)
                        ksl = bass.ts(c, 128)
                        s2 = spool.tile([128, 1024], F32)
                        nc.tensor.matmul(s2[:, 0:512], KT_pair[0:64, ksl],
                                         QT_pair[0:64, qsl], start=True, stop=True)
                        nc.tensor.matmul(s2[:, 512:1024], KT_pair[64:128, ksl],
                                         QT_pair[64:128, qsl], start=True, stop=True)
                        exp_tile(p4[:, j2].rearrange("p a b -> p (a b)"), s2[:])

                    def pv_pair(cc=cc, p4=p4, o0=o0, o1=o1, qsl=qsl):
                        st = (cc == 0)
                        sp = (cc == NPAIR - 1)
                        nc.tensor.matmul(o0[:], V_aug[0][:, cc, :, 0:65],
                                         p4[:, :, 0, :], start=st, stop=sp,
                                         perf_mode=DR)
                        nc.tensor.matmul(o1[:], V_aug[1][:, cc, :, 0:65],
                                         p4[:, :, 1, :], start=st, stop=sp,
                                         perf_mode=DR)
                        if sp:
                            pending_norm.append(normalize_start(o0, 0, qsl))
                            pending_norm.append(normalize_start(o1, 1, qsl))
                    pv_q.append(pv_pair)
                    if cc >= 2:
                        pump_pv()

                # ---- head h2: 16 chunk-pairs (even/odd in the row groups) ----
                o2 = opool.tile([65, 512], F32, tag="o", name=f"o2_{jq}")
                for cc in range(NPAIR):
                    ce, co = 2 * cc, 2 * cc + 1
                    s2 = spool.tile([128, 1024], F32)
                    nc.tensor.matmul(s2[:, 0:512], KT_h2[0:64, bass.ts(ce, 128)],
                                     QT_h2[0:64, qsl], start=True, stop=True)
                    nc.tensor.matmul(s2[:, 512:1024], KT_h2[64:128, bass.ts(co, 128)],
                                     QT_h2[64:128, qsl], start=True, stop=True)
                    p2h = ppool.tile([128, 2, 512], FP8, tag="p2h",
                                     name=f"p2h_{jq}_{cc}")
                    exp_tile(p2h[:].rearrange("p a b -> p (a b)"), s2[:])

                    def pv_h2(cc=cc, p2h=p2h, o2=o2, qsl=qsl):
                        st = (cc == 0)
                        sp = (cc == NPAIR - 1)
                        nc.tensor.matmul(o2[:], V_aug[2][:, cc, :, 0:65],
                                         p2h[:], start=st, stop=sp, perf_mode=DR)
                        if sp:
                            pending_norm.append(normalize_start(o2, 2, qsl))
                    pv_q.append(pv_h2)
                    pump_pv()

                pending.append(jq)

            pump_pv(len(pv_q))
            while pending_norm:
                normalize_finish(pending_norm.pop(0))
            for pj in pending:
                for t4 in range(4):
                    proj_subtile(pj, t4)

    nc.compile()
    return nc


_PROGRAM = None


def _get_program():
    global _PROGRAM
    if _PROGRAM is None:
        _PROGRAM = build_program()
    return _PROGRAM


def make_core_inputs(x, W_qkv, b_qkv, W_proj):
    """Per-core input dicts implementing the (batch, head-group) sharding."""
    x = np.ascontiguousarray(np.asarray(x, np.float32))
    W_qkv = np.asarray(W_qkv, np.float32)
    b_qkv = np.asarray(b_qkv, np.float32)
    W_proj = np.asarray(W_proj, np.float32)
    bf = ml_dtypes.bfloat16

    ins = []
    for c in range(NC):
        b, g = divmod(c, 4)
        col = 192 * g
        wgq = np.zeros((6, D, 128), np.float32)
        bias = np.zeros((128, 8), np.float32)
        for i, off in enumerate((0, D, 2 * D)):  # q, k, v blocks of W_qkv
            wgq[i] = W_qkv[:, off + col: off + col + 128]
            bias[:, i] = b_qkv[off + col: off + col + 128]
        q2 = W_qkv[:, col + 128: col + 192]
        k2 = W_qkv[:, D + col + 128: D + col + 192]
        v2 = W_qkv[:, 2 * D + col + 128: 2 * D + col + 192]
        wgq[3] = np.concatenate([q2, q2], axis=1)
        wgq[4] = np.concatenate([k2, k2], axis=1)
        wgq[5, :, 0:64] = v2
        bias[0:64, 3] = b_qkv[col + 128: col + 192]
        bias[64:128, 3] = bias[0:64, 3]
        bias[0:64, 4] = b_qkv[D + col + 128: D + col + 192]
        bias[64:128, 4] = bias[0:64, 4]
        bias[0:64, 5] = b_qkv[2 * D + col + 128: 2 * D + col + 192]

        wgt = np.ascontiguousarray(
            wgq.reshape(6, KCH, 128, 128).transpose(2, 0, 1, 3)
        )  # [g, (c p), m] -> [p, g, c, m]
        ins.append({
            "xb": np.ascontiguousarray(x[b]).astype(bf),
            "wg": wgt.astype(bf),
            "bias": bias,
            "wpp": W_proj[col: col + 128, :].astype(bf),
            "wp2": W_proj[col + 128: col + 192, :].astype(bf),
        })
    return ins


def gather_output(results, b_proj):
    b_proj = np.asarray(b_proj, np.float32)
    outs = []
    for b in range(B):
        acc = results[4 * b]["y"].astype(np.float32).copy()
        for c in range(4 * b + 1, 4 * b + 4):
            acc += results[c]["y"]
        outs.append(acc + b_proj)
    return np.stack(outs).astype(np.float32)


def kernel(x, W_qkv, b_qkv, W_proj, b_proj):
    ins = make_core_inputs(x, W_qkv, b_qkv, W_proj)
    prog = _get_program()
    res = run_bass_kernel_spmd(prog, ins, core_ids=list(range(NC)))
    return gather_output(res.results, b_proj)


# revision 21
# speedup vs baseline: 1.0069x; 1.0016x over previous
"""Multi-head attention (B=2, N=4096, D=768, H=12) on 8 Trainium2 NeuronCores.

Sharding: core c handles batch b = c//4 and heads [3g, 3g+1, 3g+2] with
g = c%4 (data parallel on B, head parallel on H). Each core computes its
heads' Q/K/V from x[b], runs softmax attention, and produces the partial
output projection for its head block; the host sums the 4 partials per
batch (row-parallel unshard) and adds b_proj.

Device kernel (v4 — fp8 probs, DoubleRow PV, dual-engine exp):
  - x is cast to bf16 and transposed via the DMA xbar into x^T chunks
    (x loads on the HWDGE scalar ring; transposes on the sync ring);
    QKV projection matmuls run in bf16 with the 6 output groups
    interleaved k-outer across 6 PSUM banks so drains overlap fills.
  - scores are built as S^T[k, q] blocks in bf16; the paired heads run
    their score matmuls concurrently in separate PE row groups; the
    third head is self-paired across even/odd key chunks.
  - exp is split across the scalar engine (ACT table exp -> fp8) and
    the vector engine (Schraudolph bit trick: e4m3_bits(2^y) ~= 8*y+56
    via one tensor_scalar with int8 output viewed as fp8e4). The bit
    trick's ±7% sawtooth is zero-mean and cancels between softmax
    numerator and denominator.
  - probabilities are fp8_e4m3; PV matmuls use DoubleRow perf mode
    contracting two 128-key chunks at once (2 fp8 MACs/cell/cycle),
    halving PE time. V (fp8, ones-augmented 65th column for the
    denominator) is layed out [128, pair, 2, 80] so the pair step is
    16B-aligned.
  - PV matmuls for query tile jq are emitted lagged ~2 chunk-pairs
    behind jq's score stream, so the PE always has ready work while
    the exp engines catch up (no PE idle -> HAM stays warm).
  - 1/denom: DVE reciprocal_approx_fast on the full [65,512] tile
    (base partition 0; custom DVE ops mis-execute at base!=0 on HW);
    the normalize multiplies run on GPSIMD and are deferred into the
    next query tile so the broadcast-DMA roundtrip stalls nothing.
"""

import numpy as np
import ml_dtypes
from contextlib import ExitStack

import concourse.bass as bass
from concourse import bacc
import concourse.tile as tile
import concourse.mybir as mybir
from concourse.bass_utils import run_bass_kernel_spmd

F32 = mybir.dt.float32
BF16 = mybir.dt.bfloat16
FP8 = mybir.dt.float8e4
I16 = mybir.dt.int16
I8 = mybir.dt.int8
AF = mybir.ActivationFunctionType
ALU = mybir.AluOpType
DR = mybir.MatmulPerfMode.DoubleRow

B, N, D, H, HD = 2, 4096, 768, 12, 64
SCALE = HD ** -0.5
NC = 8
NCHUNK = N // 128          # 32 key chunks of 128
NPAIR = NCHUNK // 2        # 16 key chunk-pairs (DoubleRow granularity)
NQT = N // 512             # 8 query tiles of 512
NSC = N // 512             # 8 seq chunks of 512 (QKV stage)
KCH = D // 128             # 6 contraction chunks

# Schraudolph fast-exp to fp8_e4m3: bits(exp(s*SCALE)) ~= s*C0 + C1.
LOG2E = 1.4426950408889634
EXP_C0 = 8.0 * SCALE * LOG2E
EXP_C1 = 56.0 - 0.33       # mean-centered sawtooth

# which exp tiles go to the vector engine (pattern repeats)
DVE_EXP_FRAC_PATTERN = (False, True, False, True, False)   # 2/5 on DVE


def build_program():
    nc = bacc.Bacc("TRN2", target_bir_lowering=False, debug=False)

    xb = nc.dram_tensor("xb", [N, D], BF16, kind="ExternalInput").ap()
    wg = nc.dram_tensor("wg", [128, 6, KCH, 128], BF16, kind="ExternalInput").ap()
    bias = nc.dram_tensor("bias", [128, 8], F32, kind="ExternalInput").ap()
    wpp = nc.dram_tensor("wpp", [128, D], BF16, kind="ExternalInput").ap()
    wp2 = nc.dram_tensor("wp2", [64, D], BF16, kind="ExternalInput").ap()
    y = nc.dram_tensor("y", [N, D], F32, kind="ExternalOutput").ap()

    with tile.TileContext(nc) as tc, ExitStack() as octx:
        const = octx.enter_context(tc.tile_pool(name="const", bufs=1))
        qkpool = octx.enter_context(tc.tile_pool(name="qk", bufs=1))
        vpool = octx.enter_context(tc.tile_pool(name="vaug", bufs=1))
        opool_sb = octx.enter_context(tc.tile_pool(name="onorm", bufs=1))

        bias_sb = const.tile([128, 8], F32)
        wpp_sb = const.tile([128, D], BF16)
        wp2_sb = const.tile([64, D], BF16)
        nc.scalar.dma_start(bias_sb[:], bias)
        nc.scalar.dma_start(wpp_sb[:], wpp)
        nc.scalar.dma_start(wp2_sb[:], wp2)

        # [hd, seq] layouts; pair heads stacked on partitions 0-63 / 64-127;
        # the h2 tensors hold the same head duplicated in both halves.
        QT_pair = qkpool.tile([128, N], BF16)
        KT_pair = qkpool.tile([128, N], BF16)
        QT_h2 = qkpool.tile([128, N], BF16)
        KT_h2 = qkpool.tile([128, N], BF16)

        # V natural [seq, hd] per head in fp8, DoubleRow pair layout
        # [128, pair, j, 80]: cols 0-63 = V, col 64 = ones (denominator),
        # cols 65-79 padding so the j step (80B) is 16B-aligned.
        V_aug = [
            vpool.tile([128, NPAIR, 2, 80], FP8, tag=f"vaug{h}", name=f"vaug{h}")
            for h in range(3)
        ]
        for h in range(3):
            nc.vector.memset(V_aug[h][:, :, :, 64], 1.0)

        # O^T (normalized) [feat, seq]: pair heads stacked; h2 separate.
        O_pair = opool_sb.tile([128, N], BF16)
        O_h2 = opool_sb.tile([64, N], BF16)

        # ------------- stage A+B: x^T (bf16) and QKV projection -------------
        with ExitStack() as bctx:
            wpool = bctx.enter_context(tc.tile_pool(name="wqkv", bufs=1))
            xpool = bctx.enter_context(tc.tile_pool(name="xin", bufs=4))
            xtpool = bctx.enter_context(tc.tile_pool(name="xT", bufs=4))
            vtpool = bctx.enter_context(tc.tile_pool(name="vt", bufs=3))
            qkvps = bctx.enter_context(tc.tile_pool(name="qkv", bufs=6, space="PSUM"))

            wsb = wpool.tile([128, 6, KCH, 128], BF16)
            nc.scalar.dma_start(wsb[:], wg)

            def make_xT(j):
                # xT_j[p, t, k, m] = x4[m, t, 128k+p]: one contiguous xbar
                # transpose per chunk (strided xbar outputs mis-write on HW);
                # x arrives bf16 from the host so no cast pass is needed
                xT_j = xtpool.tile([128, 4, KCH, 128], BF16, tag="xT", name=f"xT_{j}")
                x4 = xpool.tile([128, 4, D], BF16, tag="x_t", name=f"x_{j}")
                nc.gpsimd.dma_start(
                    x4[:],
                    xb[512 * j: 512 * (j + 1), :].rearrange("(t p) d -> p t d", p=128),
                )
                nc.sync.dma_start_transpose(
                    xT_j[:].rearrange("p t k m -> p (t k) m"),
                    x4[:].rearrange("p t d -> p (t d)"),
                )
                return xT_j

            def qkv_chunk(j, xT_j):
                jsl = bass.ts(j, 512)
                vt_p = vtpool.tile([128, 512], BF16, tag="vtp", name=f"vtp_{j}")
                vt_2 = vtpool.tile([64, 512], BF16, tag="vt2", name=f"vt2_{j}")
                # k-outer / g-inner: consecutive matmuls hit different PSUM
                # banks so each drain hides under the next matmul's fill
                pss = [
                    qkvps.tile([128, 512], F32, tag="ps", name=f"ps_{j}_{g}")
                    for g in range(6)
                ]
                for k in range(KCH):
                    for g in range(6):
                        nc.tensor.matmul(
                            pss[g][:], wsb[:, g, k, :], xT_j[:, :, k, :],
                            start=(k == 0), stop=(k == KCH - 1),
                        )
                dests = {0: QT_pair[:, jsl], 1: KT_pair[:, jsl], 3: QT_h2[:, jsl],
                         4: KT_h2[:, jsl], 2: vt_p[:]}
                for g in range(6):
                    ps = pss[g]
                    # bias-add + bf16 cast on DVE (per-partition bias scalar);
                    # keeps the scalar engine free for its DMA ring in A+B
                    if g == 5:  # single head, rows 0-63 only
                        nc.vector.tensor_scalar_add(vt_2[:], ps[0:64, :],
                                                    bias_sb[0:64, g: g + 1])
                    else:
                        nc.vector.tensor_scalar_add(dests[g], ps[:],
                                                    bias_sb[:, g: g + 1])

                # V^T -> V natural via xbar transpose (contiguous staging) then
                # strided DVE copy into the fp8 DoubleRow layout; chunks
                # c = 4j+t map to (pair, j2) = (2j + t//2, t%2) in order.
                # The pair heads transpose together in one [128,512] xbar pass.
                vstp = vtpool.tile([128, 4, 128], BF16, tag="vstp", name=f"vstp_{j}")
                nc.sync.dma_start_transpose(vstp[:], vt_p[:])
                vst2 = vtpool.tile([128, 4, 64], BF16, tag="vst2", name=f"vst2_{j}")
                nc.sync.dma_start_transpose(vst2[:], vt_2[:])
                for h, src in ((0, vstp[:, :, 0:64]), (1, vstp[:, :, 64:128]), (2, vst2[:])):
                    dst = V_aug[h][:, 2 * j: 2 * j + 2, :, 0:64]
                    nc.vector.tensor_copy(
                        dst.rearrange("p a b m -> p (a b) m"), src
                    )

            xts = {}
            for j in range(NSC):
                xts[j] = make_xT(j)
                if j >= 2:
                    qkv_chunk(j - 2, xts.pop(j - 2))
            qkv_chunk(NSC - 2, xts.pop(NSC - 2))
            qkv_chunk(NSC - 1, xts.pop(NSC - 1))

        # ---------------- stage C: attention ----------------
        with ExitStack() as cctx:
            spool = cctx.enter_context(tc.tile_pool(name="s", bufs=3, space="PSUM"))
            opool = cctx.enter_context(tc.tile_pool(name="o", bufs=2, space="PSUM"))
            ppool = cctx.enter_context(tc.tile_pool(name="p", bufs=8))
            osb_pool = cctx.enter_context(tc.tile_pool(name="osb", bufs=6))
            bcsb = cctx.enter_context(tc.tile_pool(name="bcs", bufs=4))
            rpool = cctx.enter_context(tc.tile_pool(name="r", bufs=4))
            rdpool = cctx.enter_context(tc.tile_pool(name="rd", bufs=4, space="DRAM"))
            ysb_pool = cctx.enter_context(tc.tile_pool(name="ysb", bufs=3))

            exp_idx = [0]

            def exp_tile(dst_fp8, src_ps):
                # dst_fp8: [128, 1024] fp8 view; src_ps: [128, 1024] PSUM f32
                i = exp_idx[0]
                exp_idx[0] += 1
                with nc.allow_low_precision(reason="fp8 softmax probs"):
                    if DVE_EXP_FRAC_PATTERN[i % len(DVE_EXP_FRAC_PATTERN)]:
                        nc.vector.tensor_scalar(
                            dst_fp8.bitcast(I8), src_ps, EXP_C0, EXP_C1,
                            ALU.mult, ALU.add,
                        )
                    else:
                        nc.scalar.activation(dst_fp8, src_ps, AF.Exp, scale=SCALE)

            # normalize phase 1: drain PSUM to SBUF, start 1/denom broadcast
            def normalize_start(o_ps, h, qsl):
                o_sb = osb_pool.tile([65, 512], F32)
                nc.vector.tensor_copy(o_sb[:], o_ps[:])
                # custom DVE ops mis-execute at base_partition != 0 on HW:
                # approx-reciprocal the whole [65,512] tile, use only row 64.
                r = rpool.tile([65, 512], F32)
                nc.vector.reciprocal_approx_fast(r[:], o_sb[:])
                rd = rdpool.tile([1, 512], F32)
                nc.gpsimd.dma_start(rd[:], r[64:65, :])
                bcs = bcsb.tile([64, 512], F32)
                nc.gpsimd.dma_start(bcs[:], rd[:].to_broadcast([64, 512]))
                return (o_sb, bcs, h, qsl)

            # normalize phase 2 (deferred; on the otherwise-idle GPSIMD)
            def normalize_finish(st):
                o_sb, bcs, h, qsl = st
                dest = O_pair[64 * h: 64 * (h + 1), qsl] if h < 2 else O_h2[:, qsl]
                nc.gpsimd.tensor_tensor(dest, o_sb[0:64, :], bcs[:], ALU.mult)

            def proj_subtile(pj, t4):
                # output projection of one 128-row q-subtile; borrows an s slot
                t = 4 * pj + t4
                tsl = bass.ts(t, 128)
                ysb = ysb_pool.tile([128, D], F32, tag="ysb", name=f"ysb_{t}")
                for half in range(2):
                    hsl = bass.ts(half, 384)
                    yp = spool.tile([128, 384], F32, tag="s2", name=f"yp_{t}_{half}")
                    nc.tensor.matmul(yp[:], O_pair[:, tsl], wpp_sb[:, hsl],
                                     start=True, stop=False)
                    nc.tensor.matmul(yp[:], O_h2[:, tsl], wp2_sb[:, hsl],
                                     start=False, stop=True)
                    nc.vector.tensor_copy(ysb[:, hsl], yp[:])
                nc.sync.dma_start(y[128 * t: 128 * (t + 1), :], ysb[:])

            pending = []          # query tiles awaiting projection
            pending_norm = []     # normalize finishes awaiting bcs DMA
            pv_q = []             # lagged PV matmul thunks

            def pump_pv(n=1):
                for _ in range(min(n, len(pv_q))):
                    pv_q.pop(0)()

            for jq in range(NQT):
                # drain the previous tile's lagged PV work first so its
                # normalizes are queued before this tile's pop points
                pump_pv(len(pv_q))
                qsl = bass.ts(jq, 512)
                o0 = opool.tile([65, 512], F32, tag="o", name=f"o0_{jq}")
                o1 = opool.tile([65, 512], F32, tag="o", name=f"o1_{jq}")

                # ---- heads h0/h1: 16 chunk-pairs ----
                for cc in range(NPAIR):
                    p4 = ppool.tile([128, 2, 2, 512], FP8, tag="p4",
                                    name=f"p4_{jq}_{cc}")
                    for j2 in (0, 1):
                        c = 2 * cc + j2
                        if pending_norm and c in (3, 5, 7, 9):
                            normalize_finish(pending_norm.pop(0))
                        if pending and pending[0] <= jq - 2:
                            if c in (10, 15, 20, 25):
                                # all of pj's normalizes must be emitted before
                                # its projection reads O_pair/O_h2
                                while pending_norm:
                                    normalize_finish(pending_norm.pop(0))
                                pj = pending[0]
                                proj_subtile(pj, (c - 10) // 5)
                                if c == 25:
                                    pending.pop(0)
                        ksl = bass.ts(c, 128)
                        s2 = spool.tile([128, 1024], F32)
                        nc.tensor.matmul(s2[:, 0:512], KT_pair[0:64, ksl],
                                         QT_pair[0:64, qsl], start=True, stop=True)
                        nc.tensor.matmul(s2[:, 512:1024], KT_pair[64:128, ksl],
                                         QT_pair[64:128, qsl], start=True, stop=True)
                        exp_tile(p4[:, j2].rearrange("p a b -> p (a b)"), s2[:])

                    def pv_pair(cc=cc, p4=p4, o0=o0, o1=o1, qsl=qsl):
                        st = (cc == 0)
                        sp = (cc == NPAIR - 1)
                        nc.tensor.matmul(o0[:], V_aug[0][:, cc, :, 0:65],
                                         p4[:, :, 0, :], start=st, stop=sp,
                                         perf_mode=DR)
                        nc.tensor.matmul(o1[:], V_aug[1][:, cc, :, 0:65],
                                         p4[:, :, 1, :], start=st, stop=sp,
                                         perf_mode=DR)
                        if sp:
                            pending_norm.append(normalize_start(o0, 0, qsl))
                            pending_norm.append(normalize_start(o1, 1, qsl))
                    pv_q.append(pv_pair)
                    if cc >= 2:
                        pump_pv()

                # ---- head h2: 16 chunk-pairs (even/odd in the row groups) ----
                o2 = opool.tile([65, 512], F32, tag="o", name=f"o2_{jq}")
                for cc in range(NPAIR):
                    ce, co = 2 * cc, 2 * cc + 1
                    s2 = spool.tile([128, 1024], F32)
                    nc.tensor.matmul(s2[:, 0:512], KT_h2[0:64, bass.ts(ce, 128)],
                                     QT_h2[0:64, qsl], start=True, stop=True)
                    nc.tensor.matmul(s2[:, 512:1024], KT_h2[64:128, bass.ts(co, 128)],
                                     QT_h2[64:128, qsl], start=True, stop=True)
                    p2h = ppool.tile([128, 2, 512], FP8, tag="p2h",
                                     name=f"p2h_{jq}_{cc}")
                    exp_tile(p2h[:].rearrange("p a b -> p (a b)"), s2[:])

                    def pv_h2(cc=cc, p2h=p2h, o2=o2, qsl=qsl):
                        st = (cc == 0)
                        sp = (cc == NPAIR - 1)
                        nc.tensor.matmul(o2[:], V_aug[2][:, cc, :, 0:65],
                                         p2h[:], start=st, stop=sp, perf_mode=DR)
                        if sp:
                            pending_norm.append(normalize_start(o2, 2, qsl))
                    pv_q.append(pv_h2)
                    pump_pv()

                pending.append(jq)

            pump_pv(len(pv_q))
            while pending_norm:
                normalize_finish(pending_norm.pop(0))
            for pj in pending:
                for t4 in range(4):
                    proj_subtile(pj, t4)

    nc.compile()
    return nc


_PROGRAM = None


def _get_program():
    global _PROGRAM
    if _PROGRAM is None:
        _PROGRAM = build_program()
    return _PROGRAM


def make_core_inputs(x, W_qkv, b_qkv, W_proj):
    """Per-core input dicts implementing the (batch, head-group) sharding."""
    x = np.ascontiguousarray(np.asarray(x, np.float32))
    W_qkv = np.asarray(W_qkv, np.float32)
    b_qkv = np.asarray(b_qkv, np.float32)
    W_proj = np.asarray(W_proj, np.float32)
    bf = ml_dtypes.bfloat16

    ins = []
    for c in range(NC):
        b, g = divmod(c, 4)
        col = 192 * g
        wgq = np.zeros((6, D, 128), np.float32)
        bias = np.zeros((128, 8), np.float32)
        for i, off in enumerate((0, D, 2 * D)):  # q, k, v blocks of W_qkv
            wgq[i] = W_qkv[:, off + col: off + col + 128]
            bias[:, i] = b_qkv[off + col: off + col + 128]
        q2 = W_qkv[:, col + 128: col + 192]
        k2 = W_qkv[:, D + col + 128: D + col + 192]
        v2 = W_qkv[:, 2 * D + col + 128: 2 * D + col + 192]
        wgq[3] = np.concatenate([q2, q2], axis=1)
        wgq[4] = np.concatenate([k2, k2], axis=1)
        wgq[5, :, 0:64] = v2
        bias[0:64, 3] = b_qkv[col + 128: col + 192]
        bias[64:128, 3] = bias[0:64, 3]
        bias[0:64, 4] = b_qkv[D + col + 128: D + col + 192]
        bias[64:128, 4] = bias[0:64, 4]
        bias[0:64, 5] = b_qkv[2 * D + col + 128: 2 * D + col + 192]

        wgt = np.ascontiguousarray(
            wgq.reshape(6, KCH, 128, 128).transpose(2, 0, 1, 3)
        )  # [g, (c p), m] -> [p, g, c, m]
        ins.append({
            "xb": np.ascontiguousarray(x[b]).astype(bf),
            "wg": wgt.astype(bf),
            "bias": bias,
            "wpp": W_proj[col: col + 128, :].astype(bf),
            "wp2": W_proj[col + 128: col + 192, :].astype(bf),
        })
    return ins


def gather_output(results, b_proj):
    b_proj = np.asarray(b_proj, np.float32)
    outs = []
    for b in range(B):
        acc = results[4 * b]["y"].astype(np.float32).copy()
        for c in range(4 * b + 1, 4 * b + 4):
            acc += results[c]["y"]
        outs.append(acc + b_proj)
    return np.stack(outs).astype(np.float32)


def kernel(x, W_qkv, b_qkv, W_proj, b_proj):
    ins = make_core_inputs(x, W_qkv, b_qkv, W_proj)
    prog = _get_program()
    res = run_bass_kernel_spmd(prog, ins, core_ids=list(range(NC)))
    return gather_output(res.results, b_proj)


# revision 22
# speedup vs baseline: 1.0614x; 1.0542x over previous
"""Multi-head attention (B=2, N=4096, D=768, H=12) on 8 Trainium2 NeuronCores.

Sharding: core c handles batch b = c//4 and heads [3g, 3g+1, 3g+2] with
g = c%4 (data parallel on B, head parallel on H). Each core computes its
heads' Q/K/V from x[b], runs softmax attention, and produces the partial
output projection for its head block; the host sums the 4 partials per
batch (row-parallel unshard) and adds b_proj.

Device kernel (v4 — fp8 probs, DoubleRow PV, dual-engine exp):
  - x is cast to bf16 and transposed via the DMA xbar into x^T chunks
    (x loads on the HWDGE scalar ring; transposes on the sync ring);
    QKV projection matmuls run in bf16 with the 6 output groups
    interleaved k-outer across 6 PSUM banks so drains overlap fills.
  - scores are built as S^T[k, q] blocks in bf16; the paired heads run
    their score matmuls concurrently in separate PE row groups; the
    third head is self-paired across even/odd key chunks.
  - exp is split across the scalar engine (ACT table exp -> fp8) and
    the vector engine (Schraudolph bit trick: e4m3_bits(2^y) ~= 8*y+56
    via one tensor_scalar with int8 output viewed as fp8e4). The bit
    trick's ±7% sawtooth is zero-mean and cancels between softmax
    numerator and denominator.
  - probabilities are fp8_e4m3; PV matmuls use DoubleRow perf mode
    contracting two 128-key chunks at once (2 fp8 MACs/cell/cycle),
    halving PE time. V (fp8, ones-augmented 65th column for the
    denominator) is layed out [128, pair, 2, 80] so the pair step is
    16B-aligned.
  - PV matmuls for query tile jq are emitted lagged ~2 chunk-pairs
    behind jq's score stream, so the PE always has ready work while
    the exp engines catch up (no PE idle -> HAM stays warm).
  - 1/denom: DVE reciprocal_approx_fast on the full [65,512] tile
    (base partition 0; custom DVE ops mis-execute at base!=0 on HW);
    the normalize multiplies run on GPSIMD and are deferred into the
    next query tile so the broadcast-DMA roundtrip stalls nothing.
"""

import numpy as np
import ml_dtypes
from contextlib import ExitStack

import concourse.bass as bass
from concourse import bacc
import concourse.tile as tile
import concourse.mybir as mybir
from concourse.bass_utils import run_bass_kernel_spmd

F32 = mybir.dt.float32
BF16 = mybir.dt.bfloat16
FP8 = mybir.dt.float8e4
I16 = mybir.dt.int16
I8 = mybir.dt.int8
AF = mybir.ActivationFunctionType
ALU = mybir.AluOpType
DR = mybir.MatmulPerfMode.DoubleRow

B, N, D, H, HD = 2, 4096, 768, 12, 64
SCALE = HD ** -0.5
NC = 8
NCHUNK = N // 128          # 32 key chunks of 128
NPAIR = NCHUNK // 2        # 16 key chunk-pairs (DoubleRow granularity)
NQT = N // 512             # 8 query tiles of 512
NSC = N // 512             # 8 seq chunks of 512 (QKV stage)
KCH = D // 128             # 6 contraction chunks

# Schraudolph fast-exp to fp8_e4m3: bits(exp(s*SCALE)) ~= s*C0 + C1.
LOG2E = 1.4426950408889634
EXP_C0 = 8.0 * SCALE * LOG2E
EXP_C1 = 56.0 - 0.33       # mean-centered sawtooth

# which exp tiles go to the vector engine (pattern repeats)
DVE_EXP_FRAC_PATTERN = (False, True, False, True, False, True, False)   # 3/7 on DVE


def build_program():
    nc = bacc.Bacc("TRN2", target_bir_lowering=False, debug=False)

    xb = nc.dram_tensor("xb", [N, D], BF16, kind="ExternalInput").ap()
    wg = nc.dram_tensor("wg", [128, 6, KCH, 128], BF16, kind="ExternalInput").ap()
    bias = nc.dram_tensor("bias", [128, 8], F32, kind="ExternalInput").ap()
    wpp = nc.dram_tensor("wpp", [128, D], BF16, kind="ExternalInput").ap()
    wp2 = nc.dram_tensor("wp2", [64, D], BF16, kind="ExternalInput").ap()
    y = nc.dram_tensor("y", [N, D], F32, kind="ExternalOutput").ap()

    with tile.TileContext(nc) as tc, ExitStack() as octx:
        const = octx.enter_context(tc.tile_pool(name="const", bufs=1))
        qkpool = octx.enter_context(tc.tile_pool(name="qk", bufs=1))
        vpool = octx.enter_context(tc.tile_pool(name="vaug", bufs=1))
        opool_sb = octx.enter_context(tc.tile_pool(name="onorm", bufs=1))

        bias_sb = const.tile([128, 8], F32)
        wpp_sb = const.tile([128, D], BF16)
        wp2_sb = const.tile([64, D], BF16)
        nc.scalar.dma_start(bias_sb[:], bias)
        nc.scalar.dma_start(wpp_sb[:], wpp)
        nc.scalar.dma_start(wp2_sb[:], wp2)

        # [hd, seq] layouts; pair heads stacked on partitions 0-63 / 64-127;
        # the h2 tensors hold the same head duplicated in both halves.
        QT_pair = qkpool.tile([128, N], BF16)
        KT_pair = qkpool.tile([128, N], BF16)
        QT_h2 = qkpool.tile([128, N], BF16)
        KT_h2 = qkpool.tile([128, N], BF16)

        # V natural [seq, hd] per head in fp8, DoubleRow pair layout
        # [128, pair, j, 80]: cols 0-63 = V, col 64 = ones (denominator),
        # cols 65-79 padding so the j step (80B) is 16B-aligned.
        V_aug = [
            vpool.tile([128, NPAIR, 2, 80], FP8, tag=f"vaug{h}", name=f"vaug{h}")
            for h in range(3)
        ]
        for h in range(3):
            nc.vector.memset(V_aug[h][:, :, :, 64], 1.0)

        # O^T (normalized) [feat, seq]: pair heads stacked; h2 separate.
        O_pair = opool_sb.tile([128, N], BF16)
        O_h2 = opool_sb.tile([64, N], BF16)

        # ------------- stage A+B: x^T (bf16) and QKV projection -------------
        with ExitStack() as bctx:
            wpool = bctx.enter_context(tc.tile_pool(name="wqkv", bufs=1))
            xpool = bctx.enter_context(tc.tile_pool(name="xin", bufs=4))
            xtpool = bctx.enter_context(tc.tile_pool(name="xT", bufs=4))
            vtpool = bctx.enter_context(tc.tile_pool(name="vt", bufs=3))
            qkvps = bctx.enter_context(tc.tile_pool(name="qkv", bufs=6, space="PSUM"))

            wsb = wpool.tile([128, 6, KCH, 128], BF16)
            nc.scalar.dma_start(wsb[:], wg)

            def make_xT(j):
                # xT_j[p, t, k, m] = x4[m, t, 128k+p]: one contiguous xbar
                # transpose per chunk (strided xbar outputs mis-write on HW);
                # x arrives bf16 from the host so no cast pass is needed
                xT_j = xtpool.tile([128, 4, KCH, 128], BF16, tag="xT", name=f"xT_{j}")
                x4 = xpool.tile([128, 4, D], BF16, tag="x_t", name=f"x_{j}")
                nc.gpsimd.dma_start(
                    x4[:],
                    xb[512 * j: 512 * (j + 1), :].rearrange("(t p) d -> p t d", p=128),
                )
                nc.sync.dma_start_transpose(
                    xT_j[:].rearrange("p t k m -> p (t k) m"),
                    x4[:].rearrange("p t d -> p (t d)"),
                )
                return xT_j

            def qkv_chunk(j, xT_j):
                jsl = bass.ts(j, 512)
                vt_p = vtpool.tile([128, 512], BF16, tag="vtp", name=f"vtp_{j}")
                vt_2 = vtpool.tile([64, 512], BF16, tag="vt2", name=f"vt2_{j}")
                # k-outer / g-inner: consecutive matmuls hit different PSUM
                # banks so each drain hides under the next matmul's fill
                pss = [
                    qkvps.tile([128, 512], F32, tag="ps", name=f"ps_{j}_{g}")
                    for g in range(6)
                ]
                for k in range(KCH):
                    for g in range(6):
                        nc.tensor.matmul(
                            pss[g][:], wsb[:, g, k, :], xT_j[:, :, k, :],
                            start=(k == 0), stop=(k == KCH - 1),
                        )
                dests = {0: QT_pair[:, jsl], 1: KT_pair[:, jsl], 3: QT_h2[:, jsl],
                         4: KT_h2[:, jsl], 2: vt_p[:]}
                for g in range(6):
                    ps = pss[g]
                    # bias-add + bf16 cast on DVE (per-partition bias scalar);
                    # keeps the scalar engine free for its DMA ring in A+B
                    if g == 5:  # single head, rows 0-63 only
                        nc.vector.tensor_scalar_add(vt_2[:], ps[0:64, :],
                                                    bias_sb[0:64, g: g + 1])
                    else:
                        nc.vector.tensor_scalar_add(dests[g], ps[:],
                                                    bias_sb[:, g: g + 1])

                # V^T -> V natural via xbar transpose (contiguous staging) then
                # strided DVE copy into the fp8 DoubleRow layout; chunks
                # c = 4j+t map to (pair, j2) = (2j + t//2, t%2) in order.
                # The pair heads transpose together in one [128,512] xbar pass.
                vstp = vtpool.tile([128, 4, 128], BF16, tag="vstp", name=f"vstp_{j}")
                nc.sync.dma_start_transpose(vstp[:], vt_p[:])
                vst2 = vtpool.tile([128, 4, 64], BF16, tag="vst2", name=f"vst2_{j}")
                nc.sync.dma_start_transpose(vst2[:], vt_2[:])
                for h, src in ((0, vstp[:, :, 0:64]), (1, vstp[:, :, 64:128]), (2, vst2[:])):
                    dst = V_aug[h][:, 2 * j: 2 * j + 2, :, 0:64]
                    nc.vector.tensor_copy(
                        dst.rearrange("p a b m -> p (a b) m"), src
                    )

            xts = {}
            for j in range(NSC):
                xts[j] = make_xT(j)
                if j >= 2:
                    qkv_chunk(j - 2, xts.pop(j - 2))
            qkv_chunk(NSC - 2, xts.pop(NSC - 2))
            qkv_chunk(NSC - 1, xts.pop(NSC - 1))

        # ---------------- stage C: attention ----------------
        with ExitStack() as cctx:
            spool = cctx.enter_context(tc.tile_pool(name="s", bufs=3, space="PSUM"))
            opool = cctx.enter_context(tc.tile_pool(name="o", bufs=2, space="PSUM"))
            ppool = cctx.enter_context(tc.tile_pool(name="p", bufs=8))
            osb_pool = cctx.enter_context(tc.tile_pool(name="osb", bufs=6))
            bcsb = cctx.enter_context(tc.tile_pool(name="bcs", bufs=4))
            rpool = cctx.enter_context(tc.tile_pool(name="r", bufs=4))
            rdpool = cctx.enter_context(tc.tile_pool(name="rd", bufs=4, space="DRAM"))
            ysb_pool = cctx.enter_context(tc.tile_pool(name="ysb", bufs=3))

            exp_idx = [0]

            def exp_tile(dst_fp8, src_ps):
                # dst_fp8: [128, 1024] fp8 view; src_ps: [128, 1024] PSUM f32
                i = exp_idx[0]
                exp_idx[0] += 1
                with nc.allow_low_precision(reason="fp8 softmax probs"):
                    if DVE_EXP_FRAC_PATTERN[i % len(DVE_EXP_FRAC_PATTERN)]:
                        nc.vector.tensor_scalar(
                            dst_fp8.bitcast(I8), src_ps, EXP_C0, EXP_C1,
                            ALU.mult, ALU.add,
                        )
                    else:
                        nc.scalar.activation(dst_fp8, src_ps, AF.Exp, scale=SCALE)

            # normalize phase 1: drain PSUM to SBUF, start 1/denom broadcast
            def normalize_start(o_ps, h, qsl):
                o_sb = osb_pool.tile([65, 512], F32)
                nc.vector.tensor_copy(o_sb[:], o_ps[:])
                # custom DVE ops mis-execute at base_partition != 0 on HW:
                # approx-reciprocal the whole [65,512] tile, use only row 64.
                r = rpool.tile([65, 512], F32)
                nc.vector.reciprocal_approx_fast(r[:], o_sb[:])
                rd = rdpool.tile([1, 512], F32)
                nc.gpsimd.dma_start(rd[:], r[64:65, :])
                bcs = bcsb.tile([64, 512], F32)
                nc.gpsimd.dma_start(bcs[:], rd[:].to_broadcast([64, 512]))
                return (o_sb, bcs, h, qsl)

            # normalize phase 2 (deferred; on the otherwise-idle GPSIMD)
            def normalize_finish(st):
                o_sb, bcs, h, qsl = st
                dest = O_pair[64 * h: 64 * (h + 1), qsl] if h < 2 else O_h2[:, qsl]
                nc.gpsimd.tensor_tensor(dest, o_sb[0:64, :], bcs[:], ALU.mult)

            def proj_subtile(pj, t4):
                # output projection of one 128-row q-subtile; borrows an s slot
                t = 4 * pj + t4
                tsl = bass.ts(t, 128)
                ysb = ysb_pool.tile([128, D], F32, tag="ysb", name=f"ysb_{t}")
                for half in range(2):
                    hsl = bass.ts(half, 384)
                    yp = spool.tile([128, 384], F32, tag="s2", name=f"yp_{t}_{half}")
                    nc.tensor.matmul(yp[:], O_pair[:, tsl], wpp_sb[:, hsl],
                                     start=True, stop=False)
                    nc.tensor.matmul(yp[:], O_h2[:, tsl], wp2_sb[:, hsl],
                                     start=False, stop=True)
                    nc.vector.tensor_copy(ysb[:, hsl], yp[:])
                nc.sync.dma_start(y[128 * t: 128 * (t + 1), :], ysb[:])

            pending = []          # query tiles awaiting projection
            pending_norm = []     # normalize finishes awaiting bcs DMA
            pv_q = []             # lagged PV matmul thunks

            def pump_pv(n=1):
                for _ in range(min(n, len(pv_q))):
                    pv_q.pop(0)()

            for jq in range(NQT):
                qsl = bass.ts(jq, 512)
                o0 = opool.tile([65, 512], F32, tag="o", name=f"o0_{jq}")
                o1 = opool.tile([65, 512], F32, tag="o", name=f"o1_{jq}")

                # ---- heads h0/h1: 16 chunk-pairs ----
                for cc in range(NPAIR):
                    p4 = ppool.tile([128, 2, 2, 512], FP8, tag="p4",
                                    name=f"p4_{jq}_{cc}")
                    for j2 in (0, 1):
                        c = 2 * cc + j2
                        if pending_norm and c in (3, 5, 7, 9):
                            normalize_finish(pending_norm.pop(0))
                        if pending and pending[0] <= jq - 2:
                            if c in (10, 15, 20, 25):
                                # all of pj's normalizes must be emitted before
                                # its projection reads O_pair/O_h2
                                while pending_norm:
                                    normalize_finish(pending_norm.pop(0))
                                pj = pending[0]
                                proj_subtile(pj, (c - 10) // 5)
                                if c == 25:
                                    pending.pop(0)
                        ksl = bass.ts(c, 128)
                        s2 = spool.tile([128, 1024], F32)
                        nc.tensor.matmul(s2[:, 0:512], KT_pair[0:64, ksl],
                                         QT_pair[0:64, qsl], start=True, stop=True)
                        nc.tensor.matmul(s2[:, 512:1024], KT_pair[64:128, ksl],
                                         QT_pair[64:128, qsl], start=True, stop=True)
                        exp_tile(p4[:, j2].rearrange("p a b -> p (a b)"), s2[:])

                    def pv_pair(cc=cc, p4=p4, o0=o0, o1=o1, qsl=qsl):
                        st = (cc == 0)
                        sp = (cc == NPAIR - 1)
                        nc.tensor.matmul(o0[:], V_aug[0][:, cc, :, 0:65],
                                         p4[:, :, 0, :], start=st, stop=sp,
                                         perf_mode=DR)
                        nc.tensor.matmul(o1[:], V_aug[1][:, cc, :, 0:65],
                                         p4[:, :, 1, :], start=st, stop=sp,
                                         perf_mode=DR)
                        if sp:
                            pending_norm.append(normalize_start(o0, 0, qsl))
                            pending_norm.append(normalize_start(o1, 1, qsl))
                    pv_q.append(pv_pair)
                    if cc >= 2:
                        pump_pv()

                # ---- head h2: 16 chunk-pairs (even/odd in the row groups) ----
                o2 = opool.tile([65, 512], F32, tag="o", name=f"o2_{jq}")
                for cc in range(NPAIR):
                    ce, co = 2 * cc, 2 * cc + 1
                    s2 = spool.tile([128, 1024], F32)
                    nc.tensor.matmul(s2[:, 0:512], KT_h2[0:64, bass.ts(ce, 128)],
                                     QT_h2[0:64, qsl], start=True, stop=True)
                    nc.tensor.matmul(s2[:, 512:1024], KT_h2[64:128, bass.ts(co, 128)],
                                     QT_h2[64:128, qsl], start=True, stop=True)
                    p2h = ppool.tile([128, 2, 512], FP8, tag="p2h",
                                     name=f"p2h_{jq}_{cc}")
                    exp_tile(p2h[:].rearrange("p a b -> p (a b)"), s2[:])

                    def pv_h2(cc=cc, p2h=p2h, o2=o2, qsl=qsl):
                        st = (cc == 0)
                        sp = (cc == NPAIR - 1)
                        nc.tensor.matmul(o2[:], V_aug[2][:, cc, :, 0:65],
                                         p2h[:], start=st, stop=sp, perf_mode=DR)
                        if sp:
                            pending_norm.append(normalize_start(o2, 2, qsl))
                    pv_q.append(pv_h2)
                    pump_pv()

                pending.append(jq)

            pump_pv(len(pv_q))
            while pending_norm:
                normalize_finish(pending_norm.pop(0))
            for pj in pending:
                for t4 in range(4):
                    proj_subtile(pj, t4)

    nc.compile()
    return nc


_PROGRAM = None


def _get_program():
    global _PROGRAM
    if _PROGRAM is None:
        _PROGRAM = build_program()
    return _PROGRAM


def make_core_inputs(x, W_qkv, b_qkv, W_proj):
    """Per-core input dicts implementing the (batch, head-group) sharding."""
    x = np.ascontiguousarray(np.asarray(x, np.float32))
    W_qkv = np.asarray(W_qkv, np.float32)
    b_qkv = np.asarray(b_qkv, np.float32)
    W_proj = np.asarray(W_proj, np.float32)
    bf = ml_dtypes.bfloat16

    ins = []
    for c in range(NC):
        b, g = divmod(c, 4)
        col = 192 * g
        wgq = np.zeros((6, D, 128), np.float32)
        bias = np.zeros((128, 8), np.float32)
        for i, off in enumerate((0, D, 2 * D)):  # q, k, v blocks of W_qkv
            wgq[i] = W_qkv[:, off + col: off + col + 128]
            bias[:, i] = b_qkv[off + col: off + col + 128]
        q2 = W_qkv[:, col + 128: col + 192]
        k2 = W_qkv[:, D + col + 128: D + col + 192]
        v2 = W_qkv[:, 2 * D + col + 128: 2 * D + col + 192]
        wgq[3] = np.concatenate([q2, q2], axis=1)
        wgq[4] = np.concatenate([k2, k2], axis=1)
        wgq[5, :, 0:64] = v2
        bias[0:64, 3] = b_qkv[col + 128: col + 192]
        bias[64:128, 3] = bias[0:64, 3]
        bias[0:64, 4] = b_qkv[D + col + 128: D + col + 192]
        bias[64:128, 4] = bias[0:64, 4]
        bias[0:64, 5] = b_qkv[2 * D + col + 128: 2 * D + col + 192]

        wgt = np.ascontiguousarray(
            wgq.reshape(6, KCH, 128, 128).transpose(2, 0, 1, 3)
        )  # [g, (c p), m] -> [p, g, c, m]
        ins.append({
            "xb": np.ascontiguousarray(x[b]).astype(bf),
            "wg": wgt.astype(bf),
            "bias": bias,
            "wpp": W_proj[col: col + 128, :].astype(bf),
            "wp2": W_proj[col + 128: col + 192, :].astype(bf),
        })
    return ins


def gather_output(results, b_proj):
    b_proj = np.asarray(b_proj, np.float32)
    outs = []
    for b in range(B):
        acc = results[4 * b]["y"].astype(np.float32).copy()
        for c in range(4 * b + 1, 4 * b + 4):
            acc += results[c]["y"]
        outs.append(acc + b_proj)
    return np.stack(outs).astype(np.float32)


def kernel(x, W_qkv, b_qkv, W_proj, b_proj):
    ins = make_core_inputs(x, W_qkv, b_qkv, W_proj)
    prog = _get_program()
    res = run_bass_kernel_spmd(prog, ins, core_ids=list(range(NC)))
    return gather_output(res.results, b_proj)
